# revision 29
# baseline (speedup 1.0000x reference)
"""Trainium2 Bass kernel for CloseSerializedAttn.

Computation (see reference):
  qkv = (feat @ W_qkv + b_qkv)[order]     # gather rows into serialized order
  per patch of K=128 points: dense softmax attention over 8 heads (d=32)
  out = (attn_out)[inverse] @ W_proj + b_proj

Strategy:
  - Shard the P=2048 patches over 8 cores (256 patches each). Patches are
    independent; each core indirect-DMA-gathers its feat rows from a full
    replica of feat in its HBM, computes qkv + attention + proj fused in
    SBUF/PSUM, and writes its shard of the serialized-order output
    contiguously. The host applies the final inverse scatter (cross-shard row
    permutation is not expressible on-device without all-to-all).
  - All matmul operands are bf16 (fp32 matmuls run at 1/4 PE rate); PSUM
    accumulation stays fp32, softmax denominators and the final bias-add are
    fp32. feat is converted to bf16 on host so the gather moves half the
    bytes and the PE transposes run at full rate.
  - Math folds done on host: SCALE into W_q/b_q; k-bias dropped (softmax
    row-invariant); v-bias folded into the final bias b_final = b_v@W_proj+b_proj.
  - Layouts: feat tile transposed via PE so qT/kT come out channel-major
    [d, pts] (scores matmuls need the contraction dim on partitions), v stays
    point-major [pts, d] so attention-output matmuls produce attn^T directly,
    which is exactly the lhsT the output projection needs.
  - `reps` wraps the whole patch loop in an outer hardware loop; used by the
    timing harness to measure steady-state device time by slope (the axon
    dispatch floor of ~70ms/launch otherwise swamps the ~ms device time).
"""
import math
import sys
import time

sys.path.insert(0, "/opt/trn_rl_repo")

import numpy as np
import ml_dtypes

import concourse.bass as bass
import concourse.bacc as bacc
import concourse.mybir as mybir
import concourse.tile as tile
from concourse.bass_utils import run_bass_kernel_spmd
from concourse.masks import make_identity

N, C, H, K = 262144, 256, 8, 128
D = C // H                   # 32
P_ALL = N // K               # 2048 patches
N_CORES = 8
PPC = P_ALL // N_CORES       # 256 patches per core
SCALE = 1.0 / math.sqrt(D)

F32 = mybir.dt.float32
BF16 = mybir.dt.bfloat16
I32 = mybir.dt.int32
NP_BF16 = ml_dtypes.bfloat16


def build_nc(n_patches: int, unroll: int = 8, dynamic_loop: bool = True,
             n_rows: int = N, reps: int = 1, dma_transpose: bool = False,
             v_copy_eng: str = 'vector', osb_eng: str = 'scalar',
             da_bufs: int = 2, s_bufs: int = 3, ftv_bufs: int = 2,
             interleave: bool = False, fake_gather: bool = False,
             skip_out: bool = False):
    nc = bacc.Bacc(trn_type="TRN2", name="csattn")

    feat = nc.dram_tensor("feat", [n_rows, C], BF16, kind="ExternalInput")
    idx = nc.dram_tensor("idx", [n_patches * K, 1], I32, kind="ExternalInput")
    # W_qk as lhsT blocks: [128, (c, oc) * 128] with oc in {q0,q1,k0,k1}
    wqk = nc.dram_tensor("wqk", [128, 2 * 4 * 128], BF16, kind="ExternalInput")
    wv = nc.dram_tensor("wv", [128, 2 * 256], BF16, kind="ExternalInput")
    wp = nc.dram_tensor("wp", [128, 2 * 256], BF16, kind="ExternalInput")
    bq = nc.dram_tensor("bq", [128, 2], F32, kind="ExternalInput")
    bfin = nc.dram_tensor("bfin", [128, 256], F32, kind="ExternalInput")
    out = nc.dram_tensor("out", [n_patches * K, C], BF16, kind="ExternalOutput")

    from contextlib import ExitStack
    with tile.TileContext(nc) as tc, ExitStack() as stk:
        cpool = stk.enter_context(tc.tile_pool(name="const", bufs=1))
        pool = stk.enter_context(tc.tile_pool(name="sbuf", bufs=3))
        # PSUM pools: sized to stay within 8 banks total.
        pp_ftv = stk.enter_context(tc.tile_pool(name="pp_ftv", bufs=ftv_bufs, space="PSUM"))
        pp_qk = stk.enter_context(tc.tile_pool(name="pp_qk", bufs=1, space="PSUM"))
        pp_s = stk.enter_context(
            tc.tile_pool(name="pp_s", bufs=s_bufs, space="PSUM"))
        pp_da = stk.enter_context(tc.tile_pool(name="pp_da", bufs=da_bufs, space="PSUM"))

        # --- static tiles ---
        wqk_s = cpool.tile([128, 1024], BF16)
        nc.sync.dma_start(out=wqk_s[:], in_=wqk[:, :])
        wv_s = cpool.tile([128, 512], BF16)
        nc.sync.dma_start(out=wv_s[:], in_=wv[:, :])
        wp_s = cpool.tile([128, 512], BF16)
        nc.sync.dma_start(out=wp_s[:], in_=wp[:, :])
        bq_s = cpool.tile([128, 2], F32)
        nc.sync.dma_start(out=bq_s[:], in_=bq[:, :])
        bfin_s = cpool.tile([128, 256], F32)
        nc.sync.dma_start(out=bfin_s[:], in_=bfin[:, :])
        if not dma_transpose:
            ident = cpool.tile([128, 128], BF16)
            make_identity(nc, ident[:])
        ones32 = cpool.tile([128, 32], BF16)
        nc.vector.memset(ones32[:], 1.0)

        def body(pr, idx_t, col):
            # ---- stage A: gather, featT, v per patch; qkT batched per pair ----
            v2 = []
            ftp = pool.tile([128, 512], BF16, tag="ftp", bufs=5)  # [c, j, 128]
            for j in range(2):
                g = pool.tile([128, 256], BF16, tag="g", bufs=10)
                if fake_gather:  # timing experiment only: contiguous load
                    nc.sync.dma_start(out=g[:], in_=feat[bass.ds((pr * 2 + j) * K, K), :])
                else:
                    nc.gpsimd.indirect_dma_start(
                        out=g[:],
                        out_offset=None,
                        in_=feat[:],
                        in_offset=bass.IndirectOffsetOnAxis(
                            ap=idx_t[:, col + j:col + j + 1], axis=0),
                    )

                ftp_j = ftp[:].rearrange("p (c j f) -> p c j f", c=2, j=2)[:, :, j, :]
                ftv_ps = pp_ftv.tile([128, 512], F32, tag="ftv")
                if dma_transpose:
                    # xbar transpose straight into SBUF: ftp[c, k, i] = g[i, 128k+c]
                    nc.scalar.dma_start_transpose(out=ftp_j, in_=g[:])
                else:
                    # transpose dst: upper half-bank of ftv, bitcast to bf16
                    ftT_ps = ftv_ps[:, 256:384].bitcast(BF16)
                    nc.tensor.transpose(ftT_ps[:, 0:128], g[:, 0:128], ident[:])
                    nc.tensor.transpose(ftT_ps[:, 128:256], g[:, 128:256], ident[:])
                    nc.scalar.copy(
                        ftp_j, ftT_ps[:].rearrange("p (c f) -> p c f", c=2))

                for c in range(2):
                    nc.tensor.matmul(
                        ftv_ps[:, 0:256],
                        lhsT=ftp[:, (c * 2 + j) * 128:(c * 2 + j + 1) * 128],
                        rhs=wv_s[:, c * 256:(c + 1) * 256],
                        start=(c == 0),
                        stop=(c == 1),
                    )
                v = pool.tile([128, 256], BF16, tag="v", bufs=8)
                # channel-varying bias: must be a tensor-tensor add (DVE)
                nc.vector.tensor_add(v[:], ftv_ps[:, 0:256], bfin_s[:])
                v2.append(v)

            # qkT for both patches: out [128, (oc, j) * 128], N=256 per matmul.
            # Two half-width PSUM tiles (bufs=1 ring) keep this to one bank.
            qk = pool.tile([128, 1024], BF16, tag="qkb", bufs=4)
            for half in range(2):
                qk_ps = pp_qk.tile([128, 512], F32, tag="qk")
                for oc_i in range(2):
                    oc = half * 2 + oc_i
                    for c in range(2):
                        nc.tensor.matmul(
                            qk_ps[:, oc_i * 256:(oc_i + 1) * 256],
                            lhsT=wqk_s[:, (c * 4 + oc) * 128:(c * 4 + oc + 1) * 128],
                            rhs=ftp[:, c * 256:(c + 1) * 256],
                            start=(c == 0),
                            stop=(c == 1),
                        )
                if half == 0:  # q chunks (oc 0,1): add bias
                    for c in range(2):
                        nc.vector.tensor_add(
                            qk[:, c * 256:(c + 1) * 256],
                            qk_ps[:, c * 256:(c + 1) * 256],
                            bq_s[:, c:c + 1].to_broadcast([128, 256]),
                        )
                else:
                    nc.vector.tensor_copy(qk[:, 512:1024], qk_ps[:])

            # ---- stage B: scores + exp, one PSUM tile per PE row-group ----
            # Issue order interleaves the 4 PE row-groups (hh) so their 32-row
            # subarray matmuls overlap on HW; needs all 4 s tiles live.
            s_ps4 = []
            for _hh in range(4):
                s_t = pp_s.tile([128, 512], F32, tag="s")
                s_ps4.append(s_t)
            if interleave:
                for j in range(2):
                    for ch in range(2):
                        for hh in range(4):
                            nc.tensor.matmul(
                                s_ps4[hh][:, (j * 2 + ch) * 128:(j * 2 + ch + 1) * 128],
                                lhsT=qk[32 * hh:32 * hh + 32,
                                        ((2 + ch) * 2 + j) * 128:((2 + ch) * 2 + j + 1) * 128],
                                rhs=qk[32 * hh:32 * hh + 32,
                                       (ch * 2 + j) * 128:(ch * 2 + j + 1) * 128],
                                start=True,
                                stop=True,
                                tile_position=(32 * hh, 0),
                            )
            else:
                for hh in range(4):
                    for j in range(2):
                        for ch in range(2):
                            nc.tensor.matmul(
                                s_ps4[hh][:, (j * 2 + ch) * 128:(j * 2 + ch + 1) * 128],
                                lhsT=qk[32 * hh:32 * hh + 32,
                                        ((2 + ch) * 2 + j) * 128:((2 + ch) * 2 + j + 1) * 128],
                                rhs=qk[32 * hh:32 * hh + 32,
                                       (ch * 2 + j) * 128:(ch * 2 + j + 1) * 128],
                                start=True,
                                stop=True,
                                tile_position=(32 * hh, 0),
                            )
            at2 = []
            for hh in range(4):
                at = pool.tile([128, 512], BF16, tag="at", bufs=12)
                nc.scalar.activation(at[:], s_ps4[hh][:], mybir.ActivationFunctionType.Exp)
                at2.append(at)

            # ---- stage C: denominators (merged), attn^T, projection ----
            # All 16 denominators in 4 col-tiled N=512 matmuls, written into a
            # 5th allocation of the s-ring. dn[32hh+d, (j*2+ch)*128+q] holds
            # 1/denom target for head 4ch+hh, query q, replicated over d.
            dn_ps = pp_s.tile([128, 512], F32, tag="s")
            for hh in range(4):
                nc.tensor.matmul(
                    dn_ps[32 * hh:32 * hh + 32, 0:512],
                    lhsT=ones32[:, :],
                    rhs=at2[hh][:, 0:512],
                    start=True,
                    stop=True,
                    tile_position=(0, 32 * hh),
                )
            r = pool.tile([128, 512], F32, tag="r", bufs=4)
            nc.vector.reciprocal_approx_fast(r[:], dn_ps[:])

            osb = pool.tile([128, 512], BF16, tag="osb", bufs=4)
            for j in range(2):
                da_ps = pp_da.tile([128, 512], F32, tag="da")
                for h in range(8):
                    hh, ch = h % 4, h // 4
                    nc.tensor.matmul(
                        da_ps[32 * hh:32 * hh + 32, ch * 128:(ch + 1) * 128],
                        lhsT=v2[j][:, 32 * h:32 * h + 32],
                        rhs=at2[hh][:, (j * 2 + ch) * 128:(j * 2 + ch + 1) * 128],
                        start=True,
                        stop=True,
                        tile_position=(0, 32 * hh),
                    )
                attn = pool.tile([128, 256], BF16, tag="attn", bufs=6)
                nc.vector.tensor_mul(attn[:], da_ps[:, 0:256],
                                     r[:, j * 256:(j + 1) * 256])

                # projection accumulates into the other half-bank of da_ps
                for c in range(2):
                    nc.tensor.matmul(
                        da_ps[:, 256:512],
                        lhsT=attn[:, c * 128:(c + 1) * 128],
                        rhs=wp_s[:, c * 256:(c + 1) * 256],
                        start=(c == 0),
                        stop=(c == 1),
                    )
                # stage PSUM->SBUF (bf16 cast) off-DVE; bias already in v
                # NOTE: gpsimd (Pool) has no PSUM port - crashes the device
                if osb_eng == 'scalar':
                    nc.scalar.copy(osb[:, j * 256:(j + 1) * 256], da_ps[:, 256:512])
                else:
                    nc.vector.tensor_copy(osb[:, j * 256:(j + 1) * 256], da_ps[:, 256:512])
            # one store per pair: HBM rows [pr*2K, pr*2K+2K) <- osb [q, (j, c)]
            if skip_out:  # timing experiment only
                return
            nc.sync.dma_start(
                out=out[bass.ds(pr * 2 * K, 2 * K), :].rearrange(
                    "(j q) c -> q j c", j=2),
                in_=osb[:])

        assert n_patches % 2 == 0

        def group_body(iv0, unroll_n):
            # one idx load per unroll-group: idx_t[i, jj] = idx[(iv0*2+jj)*K + i]
            idx_t = pool.tile([128, 2 * unroll], I32, tag="idx", bufs=3)
            nc.sync.dma_start(
                out=idx_t[:, 0:2 * unroll_n],
                in_=idx[bass.ds(iv0 * 2 * K, unroll_n * 2 * K), :].rearrange(
                    "(jj i) one -> i (jj one)", i=K),
            )
            for i in range(unroll_n):
                body(iv0 + i, idx_t, 2 * i)

        def inner_loop():
            if dynamic_loop:
                tc.For_i_unrolled_general(0, n_patches // 2, 1, group_body,
                                          max_unroll=unroll)
            else:
                n_pairs = n_patches // 2
                for g0 in range(0, n_pairs, unroll):
                    group_body(g0, min(unroll, n_pairs - g0))

        if reps == 1:
            inner_loop()
        else:
            with tc.For_i(0, reps, 1):
                inner_loop()

    nc.compile()
    return nc


def build_nc2(n_patches: int, unroll: int = 8, dynamic_loop: bool = True,
              n_rows: int = N, reps: int = 1,
              pair_gather: bool = True, interleave: bool = False,
              interleave2: bool = False, fake_gather: bool = False,
              tr_eng: str = 'scalar', osb_eng: str = 'scalar',
              kcopy_eng: str = 'vector', idx_eng: str = 'gpsimd',
              g_bufs: int = 6, at_bufs: int = 12, ftp_bufs: int = 4,
              v_bufs: int = 4, qksb_bufs: int = 3, r_bufs: int = 3,
              attn_bufs: int = 3, osb_bufs: int = 3):
    """Redesigned kernel:
      - idx loads issued on gpsimd (Pool) so the gather stream never waits on
        compute (kills the unroll-group boundary pipeline drain).
      - one indirect gather per PAIR ([128,2] offset AP) when pair_gather.
      - single rotating PSUM ring (8 x [128,512] f32 banks, one tag) so all
        stage tiles time-multiplex the 8 banks; scores matmuls issue
        interleaved across PE row-groups for subarray concurrency.
      - elementwise ops batched to [128,512]: 1 transpose-stage copy, 1 v-bias
        add, 1 q-bias add (host-precomputed [128,512] bias), 1 k copy,
        1 attn-mul, 1 osb copy per pair.
    """
    nc = bacc.Bacc(trn_type="TRN2", name="csattn2")

    feat = nc.dram_tensor("feat", [n_rows, C], BF16, kind="ExternalInput")
    # idx pre-transposed on host: idx[i, p] = order[p*K + i]
    idx = nc.dram_tensor("idx", [128, n_patches], I32, kind="ExternalInput")
    wqk = nc.dram_tensor("wqk", [128, 2 * 4 * 128], BF16, kind="ExternalInput")
    wv = nc.dram_tensor("wv", [128, 2 * 256], BF16, kind="ExternalInput")
    wp = nc.dram_tensor("wp", [128, 2 * 256], BF16, kind="ExternalInput")
    bq2 = nc.dram_tensor("bq2", [128, 512], F32, kind="ExternalInput")
    bfin2 = nc.dram_tensor("bfin2", [128, 512], F32, kind="ExternalInput")
    out = nc.dram_tensor("out", [n_patches * K, C], BF16, kind="ExternalOutput")

    from contextlib import ExitStack
    with tile.TileContext(nc) as tc, ExitStack() as stk:
        cpool = stk.enter_context(tc.tile_pool(name="const", bufs=1))
        pool = stk.enter_context(tc.tile_pool(name="sbuf", bufs=3))
        # PSUM pools, 8 banks total. interleave=False: a3/qk2/s2/c1;
        # interleave=True: a2/qk1/s4/c1 (all 4 score tiles live at once).
        a_bufs, qk_bufs, s_bufs = (2, 1, 4) if interleave else (3, 2, 2)
        pp_a = stk.enter_context(tc.tile_pool(name="pp_a", bufs=a_bufs, space="PSUM"))
        pp_qk = stk.enter_context(tc.tile_pool(name="pp_qk", bufs=qk_bufs, space="PSUM"))
        pp_s = stk.enter_context(tc.tile_pool(name="pp_s", bufs=s_bufs, space="PSUM"))
        pp_c = stk.enter_context(tc.tile_pool(name="pp_c", bufs=1, space="PSUM"))

        # --- static tiles ---
        wqk_s = cpool.tile([128, 1024], BF16)
        nc.sync.dma_start(out=wqk_s[:], in_=wqk[:, :])
        wv_s = cpool.tile([128, 512], BF16)
        nc.sync.dma_start(out=wv_s[:], in_=wv[:, :])
        wp_s = cpool.tile([128, 512], BF16)
        nc.sync.dma_start(out=wp_s[:], in_=wp[:, :])
        bq2_s = cpool.tile([128, 512], F32)
        nc.sync.dma_start(out=bq2_s[:], in_=bq2[:, :])
        bfin2_s = cpool.tile([128, 512], F32)
        nc.sync.dma_start(out=bfin2_s[:], in_=bfin2[:, :])
        ident = cpool.tile([128, 128], BF16)
        make_identity(nc, ident[:])
        ones32 = cpool.tile([128, 32], BF16)
        nc.vector.memset(ones32[:], 1.0)

        def copy_engine(which):
            return {"scalar": nc.scalar, "vector": nc.vector}[which]

        def body(pr, idx_t, col):
            # ---- stage A: gather pair, transpose, v; qk batched ----
            g = pool.tile([128, 512], BF16, tag="g", bufs=g_bufs)
            if fake_gather:  # timing diagnostic only: contiguous load
                nc.gpsimd.dma_start(
                    out=g[:], in_=feat[bass.ds(pr * 2 * K, 2 * K), :].rearrange(
                        "(j q) c -> q (j c)", j=2))
            elif pair_gather:
                nc.gpsimd.indirect_dma_start(
                    out=g[:].rearrange("p (j f) -> p j f", j=2),
                    out_offset=None,
                    in_=feat[:],
                    in_offset=bass.IndirectOffsetOnAxis(
                        ap=idx_t[:, col:col + 2], axis=0),
                )
            else:
                for j in range(2):
                    nc.gpsimd.indirect_dma_start(
                        out=g[:, j * 256:(j + 1) * 256],
                        out_offset=None,
                        in_=feat[:],
                        in_offset=bass.IndirectOffsetOnAxis(
                            ap=idx_t[:, col + j:col + j + 1], axis=0),
                    )

            # transposes: tr bf16 view [128, 4*128]; block b=(j*2+c) = g(j,c).T
            tr_ps = pp_a.tile([128, 512], F32, tag="a", name="tr_ps")
            tr_bf = tr_ps[:, 0:256].bitcast(BF16)
            for j in range(2):
                for c in range(2):
                    b = j * 2 + c
                    nc.tensor.transpose(
                        tr_bf[:, b * 128:(b + 1) * 128],
                        g[:, b * 128:(b + 1) * 128], ident[:])
            ftp = pool.tile([128, 512], BF16, tag="ftp", bufs=ftp_bufs)
            if tr_eng == 'scalar':
                nc.scalar.copy(ftp[:], tr_bf[:])
            else:
                nc.vector.tensor_copy(ftp[:], tr_bf[:])

            # v for both patches: ftv cols = (j, vch)
            ftv_ps = pp_a.tile([128, 512], F32, tag="a", name="ftv_ps")
            for j in range(2):
                for c in range(2):
                    nc.tensor.matmul(
                        ftv_ps[:, j * 256:(j + 1) * 256],
                        lhsT=ftp[:, (j * 2 + c) * 128:(j * 2 + c + 1) * 128],
                        rhs=wv_s[:, c * 256:(c + 1) * 256],
                        start=(c == 0),
                        stop=(c == 1),
                    )
            v_sb = pool.tile([128, 512], BF16, tag="v", bufs=v_bufs)
            nc.vector.tensor_add(v_sb[:], ftv_ps[:], bfin2_s[:])

            # qkT: qk_sb [128, (oc, j, f)], oc in {q0,q1,k0,k1}
            qk = pool.tile([128, 1024], BF16, tag="qkb", bufs=qksb_bufs)
            ftp_c = ftp[:].rearrange("p (j c f) -> p c j f", j=2, c=2)
            for half in range(2):
                qk_ps = pp_qk.tile([128, 512], F32, tag="qk", name="qk_ps")
                for oc_i in range(2):
                    oc = half * 2 + oc_i
                    for c in range(2):
                        nc.tensor.matmul(
                            qk_ps[:, oc_i * 256:(oc_i + 1) * 256],
                            lhsT=wqk_s[:, (c * 4 + oc) * 128:(c * 4 + oc + 1) * 128],
                            rhs=ftp_c[:, c],
                            start=(c == 0),
                            stop=(c == 1),
                        )
                if half == 0:
                    nc.vector.tensor_add(qk[:, 0:512], qk_ps[:], bq2_s[:])
                elif kcopy_eng == 'vector':
                    nc.vector.tensor_copy(qk[:, 512:1024], qk_ps[:])
                else:
                    nc.scalar.copy(qk[:, 512:1024], qk_ps[:])

            # ---- stage B: scores + exp ----
            def s_mm(s_t, hh, j, ch):
                nc.tensor.matmul(
                    s_t[:, (j * 2 + ch) * 128:(j * 2 + ch + 1) * 128],
                    lhsT=qk[32 * hh:32 * hh + 32,
                            ((2 + ch) * 2 + j) * 128:((2 + ch) * 2 + j + 1) * 128],
                    rhs=qk[32 * hh:32 * hh + 32,
                           (ch * 2 + j) * 128:(ch * 2 + j + 1) * 128],
                    start=True,
                    stop=True,
                    tile_position=(32 * hh, 0),
                )

            at2 = []
            if interleave:
                s_ps4 = [pp_s.tile([128, 512], F32, tag="s", name=f"s_ps{hh}")
                         for hh in range(4)]
                for j in range(2):
                    for ch in range(2):
                        for hh in range(4):
                            s_mm(s_ps4[hh], hh, j, ch)
                for hh in range(4):
                    at = pool.tile([128, 512], BF16, tag="at", bufs=at_bufs)
                    nc.scalar.activation(at[:], s_ps4[hh][:],
                                         mybir.ActivationFunctionType.Exp)
                    at2.append(at)
            elif interleave2:
                # 2-way row-group interleave; only 2 s tiles live (fits the
                # cfgA s-ring of 2).
                for hp in range(2):          # row-group pairs (0,1), (2,3)
                    s_a = pp_s.tile([128, 512], F32, tag="s", name=f"s_ps{2*hp}")
                    s_b = pp_s.tile([128, 512], F32, tag="s", name=f"s_ps{2*hp+1}")
                    for j in range(2):
                        for ch in range(2):
                            s_mm(s_a, 2 * hp, j, ch)
                            s_mm(s_b, 2 * hp + 1, j, ch)
                    for s_t in (s_a, s_b):
                        at = pool.tile([128, 512], BF16, tag="at", bufs=at_bufs)
                        nc.scalar.activation(at[:], s_t[:],
                                             mybir.ActivationFunctionType.Exp)
                        at2.append(at)
            else:
                for hh in range(4):
                    s_t = pp_s.tile([128, 512], F32, tag="s", name=f"s_ps{hh}")
                    for j in range(2):
                        for ch in range(2):
                            s_mm(s_t, hh, j, ch)
                    at = pool.tile([128, 512], BF16, tag="at", bufs=at_bufs)
                    nc.scalar.activation(at[:], s_t[:],
                                         mybir.ActivationFunctionType.Exp)
                    at2.append(at)

            # ---- stage C: denominators, attn, projection ----
            dn_ps = pp_s.tile([128, 512], F32, tag="s", name="dn_ps")
            for hh in range(4):
                nc.tensor.matmul(
                    dn_ps[32 * hh:32 * hh + 32, 0:512],
                    lhsT=ones32[:, :],
                    rhs=at2[hh][:, 0:512],
                    start=True,
                    stop=True,
                    tile_position=(0, 32 * hh),
                )
            r = pool.tile([128, 512], F32, tag="r", bufs=r_bufs)
            nc.vector.reciprocal_approx_fast(r[:], dn_ps[:])

            # attn-out^T for both patches: da2 cols = (j, ch, q)
            da2_ps = pp_c.tile([128, 512], F32, tag="c", name="da2_ps")
            for j in range(2):
                for h in range(8):
                    hh, ch = h % 4, h // 4
                    nc.tensor.matmul(
                        da2_ps[32 * hh:32 * hh + 32,
                               j * 256 + ch * 128:j * 256 + (ch + 1) * 128],
                        lhsT=v_sb[:, j * 256 + 32 * h:j * 256 + 32 * h + 32],
                        rhs=at2[hh][:, (j * 2 + ch) * 128:(j * 2 + ch + 1) * 128],
                        start=True,
                        stop=True,
                        tile_position=(0, 32 * hh),
                    )
            attn = pool.tile([128, 512], BF16, tag="attn", bufs=attn_bufs)
            nc.vector.tensor_mul(attn[:], da2_ps[:], r[:])

            # projection: proj cols = (j, outch)
            proj_ps = pp_c.tile([128, 512], F32, tag="c", name="proj_ps")
            for j in range(2):
                for ch in range(2):
                    nc.tensor.matmul(
                        proj_ps[:, j * 256:(j + 1) * 256],
                        lhsT=attn[:, j * 256 + ch * 128:j * 256 + (ch + 1) * 128],
                        rhs=wp_s[:, ch * 256:(ch + 1) * 256],
                        start=(ch == 0),
                        stop=(ch == 1),
                    )
            osb = pool.tile([128, 512], BF16, tag="osb", bufs=osb_bufs)
            if osb_eng == 'scalar':
                nc.scalar.copy(osb[:], proj_ps[:])
            else:
                nc.vector.tensor_copy(osb[:], proj_ps[:])
            nc.sync.dma_start(
                out=out[bass.ds(pr * 2 * K, 2 * K), :].rearrange(
                    "(j q) c -> q j c", j=2),
                in_=osb[:])

        assert n_patches % 2 == 0

        def group_body(iv0, unroll_n):
            # idx load on gpsimd (SWDGE): Pool program order is idx loads and
            # gathers only, so it runs ahead of compute freely.
            idx_t = pool.tile([128, 2 * unroll], I32, tag="idx", bufs=3)
            idx_issuer = {"gpsimd": nc.gpsimd, "sync": nc.sync}[idx_eng]
            idx_issuer.dma_start(
                out=idx_t[:, 0:2 * unroll_n],
                in_=idx[:, bass.ds(iv0 * 2, unroll_n * 2)],
            )
            for i in range(unroll_n):
                body(iv0 + i, idx_t, 2 * i)

        def inner_loop():
            if dynamic_loop:
                tc.For_i_unrolled_general(0, n_patches // 2, 1, group_body,
                                          max_unroll=unroll)
            else:
                n_pairs = n_patches // 2
                for g0 in range(0, n_pairs, unroll):
                    group_body(g0, min(unroll, n_pairs - g0))

        if reps == 1:
            inner_loop()
        else:
            with tc.For_i(0, reps, 1):
                inner_loop()

    nc.compile()
    return nc


def prep_weights(W_qkv, b_qkv, W_proj, b_proj):
    """Weight/bias folds shared by the real kernel and local checks."""
    W_qkv = np.asarray(W_qkv, dtype=np.float32)
    b_qkv = np.asarray(b_qkv, dtype=np.float32)
    W_proj = np.asarray(W_proj, dtype=np.float32)
    b_proj = np.asarray(b_proj, dtype=np.float32)

    Wq = W_qkv[:, 0:C] * SCALE          # fold attention scale into q
    Wk = W_qkv[:, C:2 * C]
    Wv = W_qkv[:, 2 * C:3 * C]
    bqv = b_qkv[0:C] * SCALE
    bv = b_qkv[2 * C:3 * C]

    # wqk blocks: index (c*4 + oc): lhsT block [C-chunk c, out-chunk oc]
    # oc 0,1 -> q chunks; oc 2,3 -> k chunks
    Wqk = np.concatenate([Wq, Wk], axis=1)  # [256, 512]
    blocks = []
    for c in range(2):
        for oc in range(4):
            blocks.append(Wqk[c * 128:(c + 1) * 128, oc * 128:(oc + 1) * 128])
    wqk_host = np.concatenate(blocks, axis=1).astype(NP_BF16)  # [128, 1024]

    wv_host = Wv.reshape(2, 128, 256).transpose(1, 0, 2).reshape(128, 512).astype(NP_BF16)
    wp_host = W_proj.reshape(2, 128, 256).transpose(1, 0, 2).reshape(128, 512).astype(NP_BF16)
    bq_host = bqv.reshape(2, 128).T.copy()  # [128, 2] fp32
    # v-bias refold: adding beta to every v row adds beta @ W_proj to every
    # output row (softmax rows sum to 1), so beta covers b_v and b_proj.
    if np.any(b_proj):
        beta = bv + np.linalg.solve(W_proj.T, b_proj)
    else:
        beta = bv
    bfin_host = np.broadcast_to(beta.astype(np.float32), (128, 256)).copy()
    return {
        "wqk": wqk_host,
        "wv": np.ascontiguousarray(wv_host),
        "wp": np.ascontiguousarray(wp_host),
        "bq": bq_host,
        "bfin": bfin_host,
    }


def prep_host_inputs_v1(feat, W_qkv, b_qkv, W_proj, b_proj, order):
    """v1 per-core input maps (idx as [PPC*K, 1] column)."""
    feat = np.asarray(feat, dtype=np.float32).astype(NP_BF16)
    order = np.asarray(order)
    weights = prep_weights(W_qkv, b_qkv, W_proj, b_proj)

    order32 = order.astype(np.int32).reshape(-1, 1)
    in_maps = []
    for i in range(N_CORES):
        in_maps.append({
            "feat": feat,
            "idx": np.ascontiguousarray(order32[i * PPC * K:(i + 1) * PPC * K]),
            **weights,
        })
    return in_maps


def prep_host_inputs(feat, W_qkv, b_qkv, W_proj, b_proj, order):
    """v2 per-core input maps: idx pre-transposed to [128, n_patches];
    bq/bfin pre-broadcast to [128, 512]."""
    feat = np.asarray(feat, dtype=np.float32).astype(NP_BF16)
    order = np.asarray(order)
    w = prep_weights(W_qkv, b_qkv, W_proj, b_proj)

    bq2 = np.concatenate([
        np.broadcast_to(w["bq"][:, 0:1], (128, 256)),
        np.broadcast_to(w["bq"][:, 1:2], (128, 256)),
    ], axis=1).astype(np.float32)
    bfin2 = np.concatenate([w["bfin"], w["bfin"]], axis=1).astype(np.float32)
    weights = {"wqk": w["wqk"], "wv": w["wv"], "wp": w["wp"],
               "bq2": np.ascontiguousarray(bq2),
               "bfin2": np.ascontiguousarray(bfin2)}

    order32 = order.astype(np.int32)
    in_maps = []
    for i in range(N_CORES):
        sl = order32[i * PPC * K:(i + 1) * PPC * K]
        idxT = np.ascontiguousarray(sl.reshape(PPC, K).T)   # [128, PPC]
        in_maps.append({"feat": feat, "idx": idxT, **weights})
    return in_maps


_NC_CACHE = {}


def _get_nc(reps: int = 1):
    key = ("main", reps)
    if key not in _NC_CACHE:
        _NC_CACHE[key] = build_nc2(PPC, reps=reps)
    return _NC_CACHE[key]


class _PjrtRunner:
    """Compiled 8-core SPMD executable with host<->device staging split out,
    so repeated executions (for timing) don't re-transfer inputs."""

    def __init__(self, nc):
        import jax
        from jax.sharding import Mesh, PartitionSpec
        from jax.experimental.shard_map import shard_map
        from concourse import bass2jax, mybir as mb

        bass2jax.install_neuronx_cc_hook()
        self.jax = jax
        self.nc = nc
        partition_name = (
            nc.partition_id_tensor.name if nc.partition_id_tensor else None
        )
        in_names, out_names, out_avals = [], [], []
        for alloc in nc.m.functions[0].allocations:
            if not isinstance(alloc, mb.MemoryLocationSet):
                continue
            name = alloc.memorylocations[0].name
            if alloc.kind == "ExternalInput":
                if name != partition_name:
                    in_names.append(name)
            elif alloc.kind == "ExternalOutput":
                out_names.append(name)
                out_avals.append(
                    jax.core.ShapedArray(
                        tuple(alloc.tensor_shape), mb.dt.np(alloc.dtype)
                    )
                )
        self.in_names, self.out_names, self.out_avals = in_names, out_names, out_avals
        n_params, n_outs = len(in_names), len(out_avals)
        all_in_names = list(in_names) + list(out_names)
        if partition_name is not None:
            all_in_names.append(partition_name)

        def _body(*args):
            operands = list(args)
            if partition_name is not None:
                operands.append(bass2jax.partition_id_tensor())
            return tuple(
                bass2jax._bass_exec_p.bind(
                    *operands,
                    out_avals=tuple(out_avals),
                    in_names=tuple(all_in_names),
                    out_names=tuple(out_names),
                    lowering_input_output_aliases=(),
                    sim_require_finite=True,
                    sim_require_nnan=True,
                    nc=nc,
                )
            )

        self.devices = jax.devices()[:N_CORES]
        self.mesh = Mesh(np.asarray(self.devices), ("core",))
        in_specs = (PartitionSpec("core"),) * (n_params + n_outs)
        out_specs = (PartitionSpec("core"),) * n_outs
        self.sharded = jax.jit(
            shard_map(
                _body, mesh=self.mesh, in_specs=in_specs, out_specs=out_specs,
                check_rep=False,
            ),
            keep_unused=True,
        )
        self.n_params, self.n_outs = n_params, n_outs
        self.staged = None

    def stage(self, in_maps):
        """device_put concatenated per-core inputs once."""
        import jax
        from jax.sharding import NamedSharding, PartitionSpec
        sh = NamedSharding(self.mesh, PartitionSpec("core"))
        concat_in = [
            np.concatenate([np.asarray(m[name]) for m in in_maps], axis=0)
            for name in self.in_names
        ]
        self.staged = [jax.device_put(a, sh) for a in concat_in]
        self.zero_shapes = [
            (N_CORES * av.shape[0], *av.shape[1:]) for av in self.out_avals
        ]
        self.zero_dtypes = [av.dtype for av in self.out_avals]
        self.sh = sh
        jax.block_until_ready(self.staged)

    def run(self):
        import jax
        import jax.numpy as jnp
        zeros = [
            jax.device_put(jnp.zeros(s, d), self.sh)
            for s, d in zip(self.zero_shapes, self.zero_dtypes)
        ]
        jax.block_until_ready(zeros)
        t0 = time.perf_counter()
        outs = self.sharded(*self.staged, *zeros)
        outs = jax.block_until_ready(outs)
        t1 = time.perf_counter()
        self.last_wall = t1 - t0
        return {
            name: np.asarray(outs[i]).reshape(N_CORES, *self.out_avals[i].shape)
            for i, name in enumerate(self.out_names)
        }


_RUNNER_CACHE = {}


def _get_runner(reps: int = 1):
    key = ("r", reps)
    if key not in _RUNNER_CACHE:
        _RUNNER_CACHE[key] = _PjrtRunner(_get_nc(reps))
    return _RUNNER_CACHE[key]


def kernel(feat, W_qkv, b_qkv, W_proj, b_proj, order, inverse, _timing_reps=0):
    runner = _get_runner()
    in_maps = prep_host_inputs(feat, W_qkv, b_qkv, W_proj, b_proj, order)
    runner.stage(in_maps)
    outs = runner.run()
    if _timing_reps:
        walls = [runner.last_wall]
        for _ in range(_timing_reps):
            runner.run()
            walls.append(runner.last_wall)
        kernel._walls = walls
    ser = outs["out"].reshape(N, C).astype(np.float32)
    final = np.empty((N, C), dtype=np.float32)
    final[np.asarray(order)] = ser
    return final

